# revision 13
# baseline (speedup 1.0000x reference)
"""Trainium2 Bass kernel for the NeuralCDE problem.

Strategy (pure data parallel over 8 NeuronCores, 128 trajectories each):
  - All activations kept feature-major [feat, batch] in SBUF; MLP weights are
    pre-transposed on the host and used as stationary matmul operands.
  - The cubic-spline derivative dX(t) is evaluated on-device per time point:
    the three coefficient slices are DMA'd, scaled (compile-time 2*frac,
    3*frac^2 immediates), transposed via the tensor engine, and combined.
  - einsum("bhd,bd->bh", f, dX): f is computed as 16 [128,128] PSUM chunks
    (row = (h,d) pair); one big DVE multiply applies a broadcast tile
    bc[p,b] = dX[p%32, b] (built with one K=32 selection matmul); then 16
    accumulating "selection" matmuls reduce over d on the tensor engine.
  - The Heun update is folded into the same PSUM accumulation: seed matmuls
    write y (or (y+z)/2), the fb3 bias contribution is one matmul against the
    transposed coefficient stack, and the dt scaling lives in the constant
    selection matrices.  z = y + dt*k1 and y' = y + dt/2*(k1+k2) therefore
    come out of PSUM directly.

Self-contained: hardcodes B=1024, L=200, D=32, H=64, W=128, OUT=16, 8 cores.
"""

import os

import numpy as np

import concourse.bass as bass
from concourse import bacc
import concourse.mybir as mybir
import concourse.tile as tile
from concourse.bass_utils import run_bass_kernel_spmd

P = 128
B, L, D, H, W, OUT = 1024, 200, 32, 64, 128, 16
N_STEPS = 100
NCORES = 8
BC = B // NCORES  # 128 trajectories per core
NCH = (H * D) // P  # 16 chunks of f
F32 = mybir.dt.float32
AF = mybir.ActivationFunctionType
OP = mybir.AluOpType

# debug override: run fewer Heun steps (output then differs from reference)
N_RUN = int(os.environ.get("KERNEL_NSTEPS", str(N_STEPS)))

_cache = {}


class _Bacc(bacc.Bacc):
    """Bacc whose act-table chooser sees our funcs only in the one set that
    covers all of them (natural_log_exp_and_others), so a single
    InstLoadActFuncSet is emitted instead of per-softplus thrashing.
    Set ids stay aligned with act_info.json (all entries kept, in order)."""

    _ONE_SET = "natural_log_exp_and_others"
    _NEED = None  # filled lazily

    def insert_act_table_loads(self):
        import bass_rust as _bass_rust
        from concourse.hw_specs import get_activation_tables

        has_activation = any(
            isinstance(i, mybir.InstActivation)
            for b in self.main_func.blocks
            for i in b.instructions
        )
        if not has_activation:
            return
        need = {AF.Relu, AF.Abs, AF.Exp, AF.Ln, AF.Identity, AF.Copy}
        tables = []
        for name, funcs in get_activation_tables(self.m.arch).items():
            if name != self._ONE_SET:
                funcs = funcs - need
            tables.append((name, funcs))
        _bass_rust.insert_act_table_loads(self, tables)


# --------------------------------------------------------------------------
# host-side prep
# --------------------------------------------------------------------------

def _time_points(ts):
    """(dt, [(knot index, frac)] for j=0..N_STEPS) in f32, matching reference."""
    ts = np.asarray(ts, np.float32)
    t0 = ts[0]
    dt = np.float32((ts[-1] - t0) / np.float32(N_STEPS))
    pts = []
    for j in range(N_RUN + 1):
        t = np.float32(t0 + np.float32(j) * dt)
        i = int(np.clip(np.searchsorted(ts, t, side="right") - 1, 0, L - 2))
        fr = np.float32(t - ts[i])
        pts.append((i, fr))
    return dt, pts


def _const_mats(weights, dt):
    """All constant matrices, host side (float32)."""
    f32 = np.float32
    c = {}

    def padrows(m, rows=P):
        out = np.zeros((rows, m.shape[1]), f32)
        out[: m.shape[0]] = m
        return out

    # MLP weights, transposed (lhsT layout [K, M]) and zero-padded to K=128
    c["fW1T"] = padrows(weights["fW1"].T.astype(f32))       # [64->128, 128]
    c["fW2T"] = weights["fW2"].T.astype(f32).copy()          # [128, 128]
    c["fW3T"] = weights["fW3"].T.astype(f32).copy()          # [128, 2048]
    c["iW1T"] = padrows(weights["iW1"].T.astype(f32))        # [32->128, 128]
    c["iW2T"] = weights["iW2"].T.astype(f32).copy()          # [128, 128]
    c["iW3T"] = weights["iW3"].T.astype(f32).copy()          # [128, 64]
    c["dW1T"] = padrows(weights["dW1"].T.astype(f32))        # [64->128, 128]
    c["dW2T"] = weights["dW2"].T.astype(f32).copy()          # [128, 128]
    c["dW3T"] = weights["dW3"].T.astype(f32).copy()          # [128, 16]
    for n in ["fb1", "fb2", "ib1", "ib2", "db1", "db2"]:
        c[n] = weights[n].astype(f32).reshape(-1, 1).copy()  # [128, 1]
    c["ib3"] = weights["ib3"].astype(f32).reshape(-1, 1).copy()  # [64, 1]
    c["db3"] = weights["db3"].astype(f32).reshape(-1, 1).copy()  # [16, 1]

    # reduce selection matrices: S_c[p, h] = scale * [h == 4c + p//32]
    for name, scale in [("SredA", dt), ("SredB", dt * f32(0.5))]:
        S = np.zeros((P, NCH, H), f32)
        for ch in range(NCH):
            for p in range(P):
                S[p, ch, 4 * ch + p // 32] = scale
        c[name] = S.reshape(P, NCH * H)

    # fb3 folding: Fb3[32*jj + d, h] = scale * fb3[h*32+d], rows 96.. zero
    fb3 = weights["fb3"].astype(f32)
    base = fb3.reshape(H, D).T  # [32, 64]: base[d, h]
    for name, scale in [("Fb3A", dt), ("Fb3B", dt * f32(0.5))]:
        M = np.zeros((P, H), f32)
        for jj in range(3):
            M[32 * jj : 32 * jj + 32] = scale * base
        c[name] = M

    # bc combine: M0[32*jj + d, p] = [p % 32 == d], rows 96.. zero
    M0 = np.zeros((P, P), f32)
    for jj in range(3):
        for p in range(P):
            M0[32 * jj + (p % 32), p] = 1.0
    c["M0"] = M0

    # seeds
    I64 = np.zeros((P, H), f32)
    I64[:H, :H] = np.eye(H, dtype=f32)
    c["Iseed"] = I64
    c["IHalf"] = (I64 * f32(0.5)).copy()
    c["ident"] = np.eye(P, dtype=f32)
    return c


# --------------------------------------------------------------------------
# bass program
# --------------------------------------------------------------------------

def _softplus(nc, sb, out_sbuf, in_psum, bias, tag, rows=P):
    """out = softplus(in + bias); in_ is PSUM, out SBUF.

    softplus(a) = max(a,0) + ln(1 + exp(-|a|)) — exact and overflow-safe.
    Engine split keeps DVE free for the einsum multiplies:
    ACT does bias-add/exp/ln, GPSIMD does the min/max combines.
    """
    r = sb.tile([P, P], F32, tag=f"spr_{tag}", name=f"spr_{tag}")[:rows, :]
    nc.scalar.activation(r, in_psum, AF.Relu, bias=bias)
    aa = sb.tile([P, P], F32, tag=f"spa_{tag}", name=f"spa_{tag}")[:rows, :]
    nc.scalar.activation(aa, in_psum, AF.Abs, bias=bias)
    e = sb.tile([P, P], F32, tag=f"spe_{tag}", name=f"spe_{tag}")[:rows, :]
    nc.scalar.activation(e, aa, AF.Exp, scale=-1.0)
    ln = sb.tile([P, P], F32, tag=f"spl_{tag}", name=f"spl_{tag}")[:rows, :]
    nc.scalar.activation(ln, e, AF.Ln, bias=1.0)
    nc.gpsimd.tensor_tensor(out=out_sbuf, in0=r, in1=ln, op=OP.add)


def _build(nk, pts_pos, dt, pts):
    """Build the bass program. nk = number of compacted knots."""
    nc = _Bacc()

    def din(name, shape):
        return nc.dram_tensor(name, shape, F32, kind="ExternalInput")

    coefs = din("coefs", [BC, nk, 3 * D])
    x0 = din("x0", [BC, D])
    names = [
        ("fW1T", [P, P]), ("fW2T", [P, P]), ("fW3T", [P, NCH * P]),
        ("iW1T", [P, P]), ("iW2T", [P, P]), ("iW3T", [P, H]),
        ("dW1T", [P, P]), ("dW2T", [P, P]), ("dW3T", [P, OUT]),
        ("fb1", [P, 1]), ("fb2", [P, 1]), ("ib1", [P, 1]), ("ib2", [P, 1]),
        ("db1", [P, 1]), ("db2", [P, 1]), ("ib3", [H, 1]), ("db3", [OUT, 1]),
        ("SredA", [P, NCH * H]), ("SredB", [P, NCH * H]),
        ("Fb3A", [P, H]), ("Fb3B", [P, H]), ("M0", [P, P]),
        ("Iseed", [P, H]), ("IHalf", [P, H]), ("ident", [P, P]),
    ]
    dmats = {n: din(n, s) for n, s in names}
    out_d = nc.dram_tensor("out", [BC, OUT], F32, kind="ExternalOutput")

    with tile.TileContext(nc) as tc:
        with tc.tile_pool(name="consts", bufs=1) as consts, \
             tc.tile_pool(name="acts", bufs=2) as acts, \
             tc.tile_pool(name="fdp", bufs=2) as fdp, \
             tc.tile_pool(name="dxp", bufs=3) as dxp, \
             tc.tile_pool(name="pf", bufs=2, space="PSUM") as pf, \
             tc.tile_pool(name="pa", bufs=1, space="PSUM") as pa, \
             tc.tile_pool(name="pR", bufs=1, space="PSUM") as pR, \
             tc.tile_pool(name="pt", bufs=2, space="PSUM") as pt:

            # ---- load constants into SBUF ----
            cs = {}
            for n, s in names:
                t = consts.tile(s, F32, tag=f"c_{n}")
                nc.sync.dma_start(t[:], dmats[n][tuple(slice(None) for _ in s)])
                cs[n] = t

            # persistent state (feat-major [128, 128], rows H.. stay zero)
            y_t = consts.tile([P, BC], F32, tag="y_t")
            z_t = consts.tile([P, BC], F32, tag="z_t")
            nc.vector.memset(y_t[:], 0.0)
            nc.vector.memset(z_t[:], 0.0)

            # ---- dX chain for one time point ----
            def dx_chain(j):
                pos = pts_pos[j]
                _, fr = pts[j]
                s2 = float(np.float32(2.0) * fr)
                s3 = float(np.float32(3.0) * fr * fr)
                stack = dxp.tile([BC, 96], F32, tag="stack")
                nc.sync.dma_start(stack[:], coefs[:, pos, :])
                nc.gpsimd.tensor_scalar_mul(stack[:, 32:64], stack[:, 32:64], s2)
                nc.gpsimd.tensor_scalar_mul(stack[:, 64:96], stack[:, 64:96], s3)
                t1 = pt.tile([P, P], F32, tag="pt")
                nc.tensor.transpose(t1[:96, :], stack[:], cs["ident"][:])
                stackT = dxp.tile([P, BC], F32, tag="stackT")
                nc.scalar.activation(stackT[:96, :], t1[:96, :], AF.Copy)
                tb = pt.tile([P, P], F32, tag="pt")
                nc.tensor.matmul(tb[:], cs["M0"][:96, :], stackT[:96, :],
                                 start=True, stop=True)
                bc_t = dxp.tile([P, BC], F32, tag="bc")
                nc.scalar.activation(bc_t[:], tb[:], AF.Copy)
                return stackT, bc_t

            # ---- one vector-field eval + folded Heun update ----
            def eval_vf(u_t, stackT, bc_t, seeds, sred, fb3m, out_t):
                a1 = pa.tile([P, BC], F32, tag="pa")
                nc.tensor.matmul(a1[:], cs["fW1T"][:], u_t[:],
                                 start=True, stop=True)
                h1 = acts.tile([P, BC], F32, tag="h1")
                _softplus(nc, acts, h1[:], a1[:], cs["fb1"][:], "h1")
                a2 = pa.tile([P, BC], F32, tag="pa")
                nc.tensor.matmul(a2[:], cs["fW2T"][:], h1[:],
                                 start=True, stop=True)
                h2 = acts.tile([P, BC], F32, tag="h2")
                _softplus(nc, acts, h2[:], a2[:], cs["fb2"][:], "h2")

                fd = fdp.tile([P, NCH, BC], F32, tag="fd")
                for half in range(2):
                    fps = pf.tile([P, 8, BC], F32, tag="pf")
                    for c8 in range(8):
                        ch = half * 8 + c8
                        nc.tensor.matmul(
                            fps[:, c8, :],
                            cs["fW3T"][:, ch * P : (ch + 1) * P],
                            h2[:], start=True, stop=True)
                    nc.vector.tensor_tensor(
                        out=fd[:, half * 8 : (half + 1) * 8, :],
                        in0=fps[:],
                        in1=bc_t[:, None, :].to_broadcast((P, 8, BC)),
                        op=OP.mult)

                r = pR.tile([H, BC], F32, tag="pR")
                first = True
                for mat, rhs in seeds:
                    nc.tensor.matmul(r[:], mat[:], rhs[:],
                                     start=first, stop=False)
                    first = False
                nc.tensor.matmul(r[:], fb3m[:96, :], stackT[:96, :],
                                 start=False, stop=False)
                for ch in range(NCH):
                    nc.tensor.matmul(
                        r[:], sred[:, ch * H : (ch + 1) * H], fd[:, ch, :],
                        start=False, stop=(ch == NCH - 1))
                nc.scalar.activation(out_t[:H, :], r[:], AF.Copy)

            # ---- init: y0 = softplus(MLP_i(x0)) ----
            x0bm = consts.tile([BC, D], F32, tag="x0bm")
            nc.sync.dma_start(x0bm[:], x0[:, :])
            t1 = pt.tile([P, P], F32, tag="pt")
            nc.tensor.transpose(t1[:D, :], x0bm[:], cs["ident"][:])
            x0fm = consts.tile([P, BC], F32, tag="x0fm")
            nc.vector.memset(x0fm[:], 0.0)
            nc.scalar.activation(x0fm[:D, :], t1[:D, :], AF.Copy)
            ai = pa.tile([P, BC], F32, tag="pa")
            nc.tensor.matmul(ai[:], cs["iW1T"][:], x0fm[:], start=True, stop=True)
            hi1 = acts.tile([P, BC], F32, tag="h1")
            _softplus(nc, acts, hi1[:], ai[:], cs["ib1"][:], "h1")
            ai2 = pa.tile([P, BC], F32, tag="pa")
            nc.tensor.matmul(ai2[:], cs["iW2T"][:], hi1[:], start=True, stop=True)
            hi2 = acts.tile([P, BC], F32, tag="h2")
            _softplus(nc, acts, hi2[:], ai2[:], cs["ib2"][:], "h2")
            ri = pR.tile([H, BC], F32, tag="pR")
            nc.tensor.matmul(ri[:], cs["iW3T"][:], hi2[:], start=True, stop=True)
            _softplus(nc, acts, y_t[:H, :], ri[:], cs["ib3"][:], "h1", rows=H)

            # ---- Heun loop ----
            stackT_a, bc_a = dx_chain(0)
            for k in range(N_RUN):
                stackT_b, bc_b = dx_chain(k + 1)
                eval_vf(y_t, stackT_a, bc_a, [(cs["Iseed"], y_t)],
                        cs["SredA"], cs["Fb3A"], z_t)
                eval_vf(z_t, stackT_b, bc_b,
                        [(cs["IHalf"], y_t), (cs["IHalf"], z_t)],
                        cs["SredB"], cs["Fb3B"], y_t)
                stackT_a, bc_a = stackT_b, bc_b

            # ---- decoder: relu MLP + transpose out ----
            ad = pa.tile([P, BC], F32, tag="pa")
            nc.tensor.matmul(ad[:], cs["dW1T"][:], y_t[:], start=True, stop=True)
            hd1 = acts.tile([P, BC], F32, tag="h1")
            nc.scalar.activation(hd1[:], ad[:], AF.Relu, bias=cs["db1"][:])
            ad2 = pa.tile([P, BC], F32, tag="pa")
            nc.tensor.matmul(ad2[:], cs["dW2T"][:], hd1[:], start=True, stop=True)
            hd2 = acts.tile([P, BC], F32, tag="h2")
            nc.scalar.activation(hd2[:], ad2[:], AF.Relu, bias=cs["db2"][:])
            rd = pt.tile([P, P], F32, tag="pt")
            nc.tensor.matmul(rd[:OUT, :], cs["dW3T"][:], hd2[:],
                             start=True, stop=True)
            o3 = consts.tile([P, BC], F32, tag="o3")
            nc.vector.memset(o3[:], 0.0)
            nc.scalar.activation(o3[:OUT, :], rd[:OUT, :], AF.Identity,
                                 bias=cs["db3"][:])
            tf = pt.tile([P, P], F32, tag="pt")
            nc.tensor.transpose(tf[:], o3[:], cs["ident"][:])
            obm = consts.tile([BC, OUT], F32, tag="obm")
            nc.scalar.activation(obm[:], tf[:, :OUT], AF.Copy)
            nc.sync.dma_start(out_d[:, :], obm[:])

    nc.finalize()
    return nc


# --------------------------------------------------------------------------
# entry point
# --------------------------------------------------------------------------

def _prepare(**inputs):
    """Host prep: returns (nc, in_maps) for run_bass_kernel_spmd."""
    ts = np.asarray(inputs["ts"], np.float32)
    dt, pts = _time_points(ts)

    # compact the knot axis to only the used knots
    used = sorted({i for i, _ in pts})
    pos_of = {i: p for p, i in enumerate(used)}
    pts_pos = [pos_of[i] for i, _ in pts]
    nk = len(used)

    key = ("k", nk, tuple(pts_pos), float(dt),
           tuple(float(f) for _, f in pts))
    if key not in _cache:
        _cache[key] = _build(nk, pts_pos, dt, pts)
    nc = _cache[key]

    weights = {k: np.asarray(v, np.float32) for k, v in inputs.items()
               if k not in ("ts", "coef_a", "coef_b", "coef_c", "coef_d")}
    cmats = _const_mats(weights, dt)

    coefs = np.concatenate(
        [np.asarray(inputs["coef_b"], np.float32)[:, used, :],
         np.asarray(inputs["coef_c"], np.float32)[:, used, :],
         np.asarray(inputs["coef_d"], np.float32)[:, used, :]], axis=2)
    coefs = np.ascontiguousarray(coefs)
    x0 = np.ascontiguousarray(np.asarray(inputs["coef_a"], np.float32)[:, 0, :])

    in_maps = []
    for c in range(NCORES):
        sl = slice(c * BC, (c + 1) * BC)
        m = {"coefs": coefs[sl], "x0": x0[sl]}
        m.update(cmats)
        in_maps.append(m)
    return nc, in_maps


def kernel(**inputs):
    nc, in_maps = _prepare(**inputs)
    res = run_bass_kernel_spmd(nc, in_maps, core_ids=list(range(NCORES)))
    out = np.concatenate([res.results[c]["out"] for c in range(NCORES)], axis=0)
    return out.astype(np.float32)


# revision 15
# speedup vs baseline: 1.4330x; 1.4330x over previous
"""Trainium2 Bass kernel for the NeuralCDE problem.

Strategy (pure data parallel over 8 NeuronCores, 128 trajectories each):
  - All activations kept feature-major [feat, batch] in SBUF; MLP weights are
    pre-transposed on the host and used as stationary matmul operands.
  - The cubic-spline derivative dX(t) is evaluated on-device per time point:
    the three coefficient slices are DMA'd, scaled (compile-time 2*frac,
    3*frac^2 immediates), transposed via the tensor engine, and combined.
  - einsum("bhd,bd->bh", f, dX): f is computed as 16 [128,128] PSUM chunks
    (row = (h,d) pair); one big DVE multiply applies a broadcast tile
    bc[p,b] = dX[p%32, b] (built with one K=32 selection matmul); then 16
    accumulating "selection" matmuls reduce over d on the tensor engine.
  - The Heun update is folded into the same PSUM accumulation: seed matmuls
    write y (or (y+z)/2), the fb3 bias contribution is one matmul against the
    transposed coefficient stack, and the dt scaling lives in the constant
    selection matrices.  z = y + dt*k1 and y' = y + dt/2*(k1+k2) therefore
    come out of PSUM directly.

Self-contained: hardcodes B=1024, L=200, D=32, H=64, W=128, OUT=16, 8 cores.
"""

import os

import numpy as np

import concourse.bass as bass
from concourse import bacc
import concourse.mybir as mybir
import concourse.tile as tile
from concourse.bass_utils import run_bass_kernel_spmd

P = 128
B, L, D, H, W, OUT = 1024, 200, 32, 64, 128, 16
N_STEPS = 100
NCORES = 8
BC = B // NCORES  # 128 trajectories per core
NCH = (H * D) // P  # 16 chunks of f
F32 = mybir.dt.float32
AF = mybir.ActivationFunctionType
OP = mybir.AluOpType

# debug override: run fewer Heun steps (output then differs from reference)
N_RUN = int(os.environ.get("KERNEL_NSTEPS", str(N_STEPS)))
# v2 reduce: sigma-scattered state rows + 4-way column-tiled reduce matmuls
V2 = os.environ.get("KERNEL_V2", "1") == "1"

# sigma: hidden unit h lives at partition q(h); chunk c=(4p+j) row r lands
# in column-group j at partition 32j+4p+r.
QOFH = np.array([32 * ((h // 4) % 4) + 4 * (h // 16) + h % 4 for h in range(H)])

_cache = {}


class _Bacc(bacc.Bacc):
    """Bacc whose act-table chooser sees our funcs only in the one set that
    covers all of them (natural_log_exp_and_others), so a single
    InstLoadActFuncSet is emitted instead of per-softplus thrashing.
    Set ids stay aligned with act_info.json (all entries kept, in order)."""

    _ONE_SET = "natural_log_exp_and_others"
    _NEED = None  # filled lazily

    def insert_act_table_loads(self):
        import bass_rust as _bass_rust
        from concourse.hw_specs import get_activation_tables

        has_activation = any(
            isinstance(i, mybir.InstActivation)
            for b in self.main_func.blocks
            for i in b.instructions
        )
        if not has_activation:
            return
        need = {AF.Relu, AF.Abs, AF.Exp, AF.Ln, AF.Identity, AF.Copy}
        tables = []
        for name, funcs in get_activation_tables(self.m.arch).items():
            if name != self._ONE_SET:
                funcs = funcs - need
            tables.append((name, funcs))
        _bass_rust.insert_act_table_loads(self, tables)


# --------------------------------------------------------------------------
# host-side prep
# --------------------------------------------------------------------------

def _time_points(ts, n_run=None):
    """(dt, [(knot index, frac)] for j=0..n_run) in f32, matching reference."""
    if n_run is None:
        n_run = N_RUN
    ts = np.asarray(ts, np.float32)
    t0 = ts[0]
    dt = np.float32((ts[-1] - t0) / np.float32(N_STEPS))
    pts = []
    for j in range(n_run + 1):
        t = np.float32(t0 + np.float32(j) * dt)
        i = int(np.clip(np.searchsorted(ts, t, side="right") - 1, 0, L - 2))
        fr = np.float32(t - ts[i])
        pts.append((i, fr))
    return dt, pts


def _const_mats(weights, dt):
    """All constant matrices, host side (float32)."""
    f32 = np.float32
    c = {}

    def padrows(m, rows=P):
        out = np.zeros((rows, m.shape[1]), f32)
        out[: m.shape[0]] = m
        return out

    # MLP weights, transposed (lhsT layout [K, M]) and zero-padded to K=128
    c["fW1T"] = padrows(weights["fW1"].T.astype(f32))       # [64->128, 128]
    c["fW2T"] = weights["fW2"].T.astype(f32).copy()          # [128, 128]
    c["fW3T"] = weights["fW3"].T.astype(f32).copy()          # [128, 2048]
    c["iW1T"] = padrows(weights["iW1"].T.astype(f32))        # [32->128, 128]
    c["iW2T"] = weights["iW2"].T.astype(f32).copy()          # [128, 128]
    c["iW3T"] = weights["iW3"].T.astype(f32).copy()          # [128, 64]
    c["dW1T"] = padrows(weights["dW1"].T.astype(f32))        # [64->128, 128]
    c["dW2T"] = weights["dW2"].T.astype(f32).copy()          # [128, 128]
    c["dW3T"] = weights["dW3"].T.astype(f32).copy()          # [128, 16]
    for n in ["fb1", "fb2", "ib1", "ib2", "db1", "db2"]:
        c[n] = weights[n].astype(f32).reshape(-1, 1).copy()  # [128, 1]
    c["ib3"] = weights["ib3"].astype(f32).reshape(-1, 1).copy()  # [64, 1]
    c["db3"] = weights["db3"].astype(f32).reshape(-1, 1).copy()  # [16, 1]

    if V2:
        # sigma-permuted state: fW1T/dW1T rows, iW3T/ib3/Fb3 columns move to q(h)
        for wname in ("fW1T", "dW1T"):
            m = np.zeros((P, P), f32)
            m[QOFH, :] = c[wname][:H, :]
            c[wname] = m
        m = np.zeros((P, P), f32)
        m[:, QOFH] = c["iW3T"]
        c["iW3T"] = m
        bv = np.zeros((P, 1), f32)
        bv[QOFH, 0] = c["ib3"][:, 0]
        c["ib3"] = bv

    # reduce selection matrices
    if V2:
        # per-pass S_p[p', m] = scale * [m == 4p + p'//32], shared by all
        # column groups; stored [P, 4, 32]
        for name, scale in [("SredA", dt), ("SredB", dt * f32(0.5))]:
            S = np.zeros((P, 4, 32), f32)
            for pp in range(4):
                for p in range(P):
                    S[p, pp, 4 * pp + p // 32] = scale
            c[name] = S.reshape(P, 4 * 32)
    else:
        # S_c[p, h] = scale * [h == 4c + p//32]
        for name, scale in [("SredA", dt), ("SredB", dt * f32(0.5))]:
            S = np.zeros((P, NCH, H), f32)
            for ch in range(NCH):
                for p in range(P):
                    S[p, ch, 4 * ch + p // 32] = scale
            c[name] = S.reshape(P, NCH * H)

    # fb3 folding: Fb3[32*jj + d, col(h)] = scale * fb3[h*32+d], rows 96.. zero
    fb3 = weights["fb3"].astype(f32)
    base = fb3.reshape(H, D).T  # [32, 64]: base[d, h]
    MH = P if V2 else H
    for name, scale in [("Fb3A", dt), ("Fb3B", dt * f32(0.5))]:
        M = np.zeros((P, MH), f32)
        for jj in range(3):
            if V2:
                M[32 * jj : 32 * jj + 32][:, QOFH] = scale * base
            else:
                M[32 * jj : 32 * jj + 32] = scale * base
        c[name] = M

    # bc combine: M0[32*jj + d, p] = [p % 32 == d], rows 96.. zero
    M0 = np.zeros((P, P), f32)
    for jj in range(3):
        for p in range(P):
            M0[32 * jj + (p % 32), p] = 1.0
    c["M0"] = M0

    # seeds
    if V2:
        I64 = np.zeros((P, P), f32)
        I64[QOFH, QOFH] = 1.0
    else:
        I64 = np.zeros((P, H), f32)
        I64[:H, :H] = np.eye(H, dtype=f32)
    c["Iseed"] = I64
    c["IHalf"] = (I64 * f32(0.5)).copy()
    c["ident"] = np.eye(P, dtype=f32)
    return c


# --------------------------------------------------------------------------
# bass program
# --------------------------------------------------------------------------

def _softplus(nc, sb, out_sbuf, in_psum, bias, tag, rows=P):
    """out = softplus(in + bias); in_ is PSUM, out SBUF.

    softplus(a) = max(a,0) + ln(1 + exp(-|a|)) — exact and overflow-safe.
    Engine split keeps DVE free for the einsum multiplies:
    ACT does bias-add/exp/ln, GPSIMD does the min/max combines.
    """
    r = sb.tile([P, P], F32, tag=f"spr_{tag}", name=f"spr_{tag}")[:rows, :]
    nc.scalar.activation(r, in_psum, AF.Relu, bias=bias)
    aa = sb.tile([P, P], F32, tag=f"spa_{tag}", name=f"spa_{tag}")[:rows, :]
    nc.scalar.activation(aa, in_psum, AF.Abs, bias=bias)
    e = sb.tile([P, P], F32, tag=f"spe_{tag}", name=f"spe_{tag}")[:rows, :]
    nc.scalar.activation(e, aa, AF.Exp, scale=-1.0)
    ln = sb.tile([P, P], F32, tag=f"spl_{tag}", name=f"spl_{tag}")[:rows, :]
    nc.scalar.activation(ln, e, AF.Ln, bias=1.0)
    nc.gpsimd.tensor_tensor(out=out_sbuf, in0=r, in1=ln, op=OP.add)


def _build(nk, pts_pos, dt, pts):
    """Build the bass program. nk = number of compacted knots."""
    nc = _Bacc()

    def din(name, shape):
        return nc.dram_tensor(name, shape, F32, kind="ExternalInput")

    coefs = din("coefs", [BC, nk, 3 * D])
    x0 = din("x0", [BC, D])
    RM = P if V2 else H   # partition extent of the state/reduce output
    SRD = 4 * 32 if V2 else NCH * H
    names = [
        ("fW1T", [P, P]), ("fW2T", [P, P]), ("fW3T", [P, NCH * P]),
        ("iW1T", [P, P]), ("iW2T", [P, P]), ("iW3T", [P, RM]),
        ("dW1T", [P, P]), ("dW2T", [P, P]), ("dW3T", [P, OUT]),
        ("fb1", [P, 1]), ("fb2", [P, 1]), ("ib1", [P, 1]), ("ib2", [P, 1]),
        ("db1", [P, 1]), ("db2", [P, 1]), ("ib3", [RM, 1]), ("db3", [OUT, 1]),
        ("SredA", [P, SRD]), ("SredB", [P, SRD]),
        ("Fb3A", [P, RM]), ("Fb3B", [P, RM]), ("M0", [P, P]),
        ("Iseed", [P, RM]), ("IHalf", [P, RM]), ("ident", [P, P]),
    ]
    dmats = {n: din(n, s) for n, s in names}
    out_d = nc.dram_tensor("out", [BC, OUT], F32, kind="ExternalOutput")

    with tile.TileContext(nc) as tc:
        with tc.tile_pool(name="consts", bufs=1) as consts, \
             tc.tile_pool(name="acts", bufs=2) as acts, \
             tc.tile_pool(name="fdp", bufs=2) as fdp, \
             tc.tile_pool(name="dxp", bufs=3) as dxp, \
             tc.tile_pool(name="pf", bufs=2, space="PSUM") as pf, \
             tc.tile_pool(name="pa", bufs=1, space="PSUM") as pa, \
             tc.tile_pool(name="pR", bufs=1, space="PSUM") as pR, \
             tc.tile_pool(name="pt", bufs=2, space="PSUM") as pt:

            # ---- load constants into SBUF ----
            cs = {}
            for n, s in names:
                t = consts.tile(s, F32, tag=f"c_{n}")
                nc.sync.dma_start(t[:], dmats[n][tuple(slice(None) for _ in s)])
                cs[n] = t

            # persistent state (feat-major [128, 128], rows H.. stay zero)
            y_t = consts.tile([P, BC], F32, tag="y_t")
            z_t = consts.tile([P, BC], F32, tag="z_t")
            nc.vector.memset(y_t[:], 0.0)
            nc.vector.memset(z_t[:], 0.0)

            # ---- dX chain for one time point ----
            def dx_chain(j):
                pos = pts_pos[j]
                _, fr = pts[j]
                s2 = float(np.float32(2.0) * fr)
                s3 = float(np.float32(3.0) * fr * fr)
                stack = dxp.tile([BC, 96], F32, tag="stack")
                nc.sync.dma_start(stack[:], coefs[:, pos, :])
                nc.gpsimd.tensor_scalar_mul(stack[:, 32:64], stack[:, 32:64], s2)
                nc.gpsimd.tensor_scalar_mul(stack[:, 64:96], stack[:, 64:96], s3)
                t1 = pt.tile([P, P], F32, tag="pt")
                nc.tensor.transpose(t1[:96, :], stack[:], cs["ident"][:])
                stackT = dxp.tile([P, BC], F32, tag="stackT")
                nc.scalar.activation(stackT[:96, :], t1[:96, :], AF.Copy)
                tb = pt.tile([P, P], F32, tag="pt")
                nc.tensor.matmul(tb[:], cs["M0"][:96, :], stackT[:96, :],
                                 start=True, stop=True)
                bc_t = dxp.tile([P, BC], F32, tag="bc")
                nc.scalar.activation(bc_t[:], tb[:], AF.Copy)
                return stackT, bc_t

            # ---- one vector-field eval + folded Heun update ----
            def eval_vf(u_t, stackT, bc_t, seeds, sred, fb3m, out_t):
                a1 = pa.tile([P, BC], F32, tag="pa")
                nc.tensor.matmul(a1[:], cs["fW1T"][:], u_t[:],
                                 start=True, stop=True)
                h1 = acts.tile([P, BC], F32, tag="h1")
                _softplus(nc, acts, h1[:], a1[:], cs["fb1"][:], "h1")
                a2 = pa.tile([P, BC], F32, tag="pa")
                nc.tensor.matmul(a2[:], cs["fW2T"][:], h1[:],
                                 start=True, stop=True)
                h2 = acts.tile([P, BC], F32, tag="h2")
                _softplus(nc, acts, h2[:], a2[:], cs["fb2"][:], "h2")

                fd = fdp.tile([P, NCH, BC], F32, tag="fd")
                for half in range(2):
                    fps = pf.tile([P, 8, BC], F32, tag="pf")
                    for c8 in range(8):
                        ch = half * 8 + c8
                        nc.tensor.matmul(
                            fps[:, c8, :],
                            cs["fW3T"][:, ch * P : (ch + 1) * P],
                            h2[:], start=True, stop=True)
                    nc.vector.tensor_tensor(
                        out=fd[:, half * 8 : (half + 1) * 8, :],
                        in0=fps[:],
                        in1=bc_t[:, None, :].to_broadcast((P, 8, BC)),
                        op=OP.mult)

                r = pR.tile([RM, BC], F32, tag="pR")
                first = True
                for mat, rhs in seeds:
                    nc.tensor.matmul(r[:], mat[:], rhs[:],
                                     start=first, stop=False)
                    first = False
                nc.tensor.matmul(r[:], fb3m[:96, :], stackT[:96, :],
                                 start=False, stop=False)
                if V2:
                    n = 0
                    for pp in range(4):
                        for j in range(4):
                            ch = 4 * pp + j
                            n += 1
                            nc.tensor.matmul(
                                r[32 * j : 32 * j + 32, :],
                                sred[:, pp * 32 : (pp + 1) * 32],
                                fd[:, ch, :],
                                start=False, stop=(n == NCH),
                                tile_position=(0, 32 * j))
                else:
                    for ch in range(NCH):
                        nc.tensor.matmul(
                            r[:], sred[:, ch * H : (ch + 1) * H], fd[:, ch, :],
                            start=False, stop=(ch == NCH - 1))
                nc.scalar.activation(out_t[:RM, :], r[:], AF.Copy)

            # ---- init: y0 = softplus(MLP_i(x0)) ----
            x0bm = consts.tile([BC, D], F32, tag="x0bm")
            nc.sync.dma_start(x0bm[:], x0[:, :])
            t1 = pt.tile([P, P], F32, tag="pt")
            nc.tensor.transpose(t1[:D, :], x0bm[:], cs["ident"][:])
            x0fm = consts.tile([P, BC], F32, tag="x0fm")
            nc.vector.memset(x0fm[:], 0.0)
            nc.scalar.activation(x0fm[:D, :], t1[:D, :], AF.Copy)
            ai = pa.tile([P, BC], F32, tag="pa")
            nc.tensor.matmul(ai[:], cs["iW1T"][:], x0fm[:], start=True, stop=True)
            hi1 = acts.tile([P, BC], F32, tag="h1")
            _softplus(nc, acts, hi1[:], ai[:], cs["ib1"][:], "h1")
            ai2 = pa.tile([P, BC], F32, tag="pa")
            nc.tensor.matmul(ai2[:], cs["iW2T"][:], hi1[:], start=True, stop=True)
            hi2 = acts.tile([P, BC], F32, tag="h2")
            _softplus(nc, acts, hi2[:], ai2[:], cs["ib2"][:], "h2")
            ri = pR.tile([RM, BC], F32, tag="pR")
            nc.tensor.matmul(ri[:], cs["iW3T"][:], hi2[:], start=True, stop=True)
            _softplus(nc, acts, y_t[:RM, :], ri[:], cs["ib3"][:], "h1", rows=RM)

            # ---- Heun loop ----
            stackT_a, bc_a = dx_chain(0)
            for k in range(len(pts) - 1):
                stackT_b, bc_b = dx_chain(k + 1)
                eval_vf(y_t, stackT_a, bc_a, [(cs["Iseed"], y_t)],
                        cs["SredA"], cs["Fb3A"], z_t)
                eval_vf(z_t, stackT_b, bc_b,
                        [(cs["IHalf"], y_t), (cs["IHalf"], z_t)],
                        cs["SredB"], cs["Fb3B"], y_t)
                stackT_a, bc_a = stackT_b, bc_b

            # ---- decoder: relu MLP + transpose out ----
            ad = pa.tile([P, BC], F32, tag="pa")
            nc.tensor.matmul(ad[:], cs["dW1T"][:], y_t[:], start=True, stop=True)
            hd1 = acts.tile([P, BC], F32, tag="h1")
            nc.scalar.activation(hd1[:], ad[:], AF.Relu, bias=cs["db1"][:])
            ad2 = pa.tile([P, BC], F32, tag="pa")
            nc.tensor.matmul(ad2[:], cs["dW2T"][:], hd1[:], start=True, stop=True)
            hd2 = acts.tile([P, BC], F32, tag="h2")
            nc.scalar.activation(hd2[:], ad2[:], AF.Relu, bias=cs["db2"][:])
            rd = pt.tile([P, P], F32, tag="pt")
            nc.tensor.matmul(rd[:OUT, :], cs["dW3T"][:], hd2[:],
                             start=True, stop=True)
            o3 = consts.tile([P, BC], F32, tag="o3")
            nc.vector.memset(o3[:], 0.0)
            nc.scalar.activation(o3[:OUT, :], rd[:OUT, :], AF.Identity,
                                 bias=cs["db3"][:])
            tf = pt.tile([P, P], F32, tag="pt")
            nc.tensor.transpose(tf[:], o3[:], cs["ident"][:])
            obm = consts.tile([BC, OUT], F32, tag="obm")
            nc.scalar.activation(obm[:], tf[:, :OUT], AF.Copy)
            nc.sync.dma_start(out_d[:, :], obm[:])

    nc.finalize()
    return nc


# --------------------------------------------------------------------------
# entry point
# --------------------------------------------------------------------------

def _prepare(**inputs):
    """Host prep: returns (nc, in_maps) for run_bass_kernel_spmd."""
    ts = np.asarray(inputs["ts"], np.float32)
    dt, pts = _time_points(ts)

    # compact the knot axis to only the used knots
    used = sorted({i for i, _ in pts})
    pos_of = {i: p for p, i in enumerate(used)}
    pts_pos = [pos_of[i] for i, _ in pts]
    nk = len(used)

    key = ("k", nk, tuple(pts_pos), float(dt),
           tuple(float(f) for _, f in pts))
    if key not in _cache:
        _cache[key] = _build(nk, pts_pos, dt, pts)
    nc = _cache[key]

    weights = {k: np.asarray(v, np.float32) for k, v in inputs.items()
               if k not in ("ts", "coef_a", "coef_b", "coef_c", "coef_d")}
    cmats = _const_mats(weights, dt)

    coefs = np.concatenate(
        [np.asarray(inputs["coef_b"], np.float32)[:, used, :],
         np.asarray(inputs["coef_c"], np.float32)[:, used, :],
         np.asarray(inputs["coef_d"], np.float32)[:, used, :]], axis=2)
    coefs = np.ascontiguousarray(coefs)
    x0 = np.ascontiguousarray(np.asarray(inputs["coef_a"], np.float32)[:, 0, :])

    in_maps = []
    for c in range(NCORES):
        sl = slice(c * BC, (c + 1) * BC)
        m = {"coefs": coefs[sl], "x0": x0[sl]}
        m.update(cmats)
        in_maps.append(m)
    return nc, in_maps


def kernel(**inputs):
    nc, in_maps = _prepare(**inputs)
    res = run_bass_kernel_spmd(nc, in_maps, core_ids=list(range(NCORES)))
    out = np.concatenate([res.results[c]["out"] for c in range(NCORES)], axis=0)
    return out.astype(np.float32)
